# revision 21
# baseline (speedup 1.0000x reference)
"""Trainium2 Bass kernel for nn_LC_Block (gnn_message_passing).

Strategy (pure data-parallel over batch, 2 batches/core on 8 cores):
  - BN1 folded into conv1 weights; temporal conv as bf16 Toeplitz matmul on PE
    (lhsT = host-im2col'd x windows incl. a ones-row for the bias).
  - ELU via the exact identity elu(x) = max(x, min(exp(x), 1) - 1):
    ACT exp-drains PSUM, DVE computes min(e,1)-1 at 4x bf16 rate, DVE+Pool
    split the max(x, .) pass -> ONE fused elu buffer (single matmul source).
  - GCN (f-mean + a_hat propagate + affine) + residual + depthwise-expansion
    conv + BN2 folded host-side into accumulating PE matmul weights; the
    t-range is split in 256-col quarters so matmuls interleave with conv1.
  - All sigmoids via tanh (sigmoid(z) = 0.5 + 0.5 tanh(z/2)) so the whole
    kernel uses one ACT function table (exp_and_others) - no table reloads.
  - Spatial attention: c2-mean and c2-max folded directly into the 3-tap
    temporal-conv matmuls (scaled block lhs over h4 / the gpsimd all-reduce
    output) - no assembly copies or intra-SBUF DMAs.
  - Separable temporal conv as 16 accumulating diagonal matmuls on PE.
  - Inputs coalesced into 3 DMAs issued from SP/ACT/DVE hardware-DGE queues.
"""
import numpy as np
import concourse.bass as bass
import concourse.tile as tile
import concourse.mybir as mybir
import concourse.bass_isa as bass_isa
from concourse.bass_utils import run_bass_kernel_spmd

F32 = mybir.dt.float32
BF16 = mybir.dt.bfloat16
AOP = mybir.AluOpType
AFT = mybir.ActivationFunctionType

B, F1, D, C, T, K = 16, 16, 2, 64, 1000, 64
C2 = F1 * D          # 32
PW = K // 4          # 16
EPS = 1e-5
NCORE = 8
BPC = B // NCORE     # 2
TBLK = 64
NBLK = 16            # covers t 0..1023 (1000 valid)
NF = F1 * TBLK       # 1024 conv out cols per block
QSPL = 160           # DVE share of the min(e,1)-1 column split

# cb (bf16 const block) column layout
CB_WT = 0            # [128, 1024] conv1 toeplitz
CB_KS = 1024         # [128, 1024] fused gcn+depthwise weights
CB_LSAM = 2048       # [64, 6] spatial-attn mean taps (3 dt x 2 b)
CB_LSAX = 2054       # [64, 6] spatial-attn max taps
CB_LBC = 2060        # [2, 64] 0.5 * broadcast
CB_SEP = 2124        # [64, 1024] 16 diag sep-conv matrices
CBW = CB_SEP + 1024

# cf (f32 const block [64, CFW]) column layout
CF_B2H = 0           # 0.5 * bias2
CF_B2X2 = 1          # 2 * bias2
CF_BCA1 = 2          # [4] ca MLP bias 1
CF_BCA2H = 3         # 0.5 * ca MLP bias 2
CF_SSAH = 4          # [2] 0.5 * sa BN scale
CF_BSAH = 5          # [2] 0.5 * sa BN bias
CF_BSEP = 6          # sep bias (for exp path)
CF_BSEP116 = 12      # (sep bias + 1) / 16
CF_HALF = 7          # 0.5 (activation bias AP)
CF_LCA1 = 8          # [64, 4] ca MLP w1 (f32 lhsT)
CFW0 = 13            # CF_BSEP116 slot at 12
CF_LCA2 = 13         # [4, 64] ca MLP w2 (f32 lhsT)
CFW = 13 + 64


def _bf16(a):
    import ml_dtypes
    return np.asarray(a, np.float32).astype(ml_dtypes.bfloat16)


# ----------------------------------------------------------------- host prep
def _host_consts(inp):
    g1, b1, m1, v1 = (np.asarray(inp[k], np.float32) for k in ('g1', 'b1', 'm1', 'v1'))
    inv1 = g1 / np.sqrt(v1 + EPS)
    w1 = np.asarray(inp['conv1_w'], np.float32)[:, 0, 0, :]
    w1p = w1 * inv1[:, None]
    b1p = (np.asarray(inp['conv1_b'], np.float32) - m1) * inv1 + b1

    Wt = np.zeros((128, NF), np.float32)
    for toff in range(TBLK):
        for ff in range(F1):
            Wt[toff:toff + K, ff * TBLK + toff] = w1p[ff]
    Wt[127, :] = np.repeat(b1p, TBLK)

    a_hat = np.asarray(inp['a_hat'], np.float32)
    dw_w = np.asarray(inp['dw_w'], np.float32)
    gcn_w = np.asarray(inp['gcn_w'], np.float32)
    gcn_b = np.asarray(inp['gcn_b'], np.float32)
    g2, b2, m2, v2 = (np.asarray(inp[k], np.float32) for k in ('g2', 'b2', 'm2', 'v2'))
    s2 = g2 / np.sqrt(v2 + EPS)
    G = (gcn_w[:, None, None] / F1) * np.einsum('fdc,cj->fdj', dw_w, a_hat)
    Kmat = np.zeros((F1, C, C2), np.float32)          # [f', j, c2]
    for ff in range(F1):
        for d in range(D):
            Kmat[ff, :, ff * D + d] += dw_w[ff, d, :]
    for ff in range(F1):
        for d in range(D):
            Kmat[:, :, ff * D + d] += G[ff, d, :][None, :]
    Kmat *= s2[None, None, :]
    Kstk = np.zeros((128, F1, 2 * C2), np.float32)    # [(b,j), f', (b,c2)]
    for b in range(BPC):
        Kstk[b * C:(b + 1) * C, :, b * C2:(b + 1) * C2] = np.transpose(Kmat, (1, 0, 2))

    sw = dw_w.sum(-1)
    dw_b = np.asarray(inp['dw_b'], np.float32)
    bias2 = s2 * (dw_b + np.repeat(gcn_b, D) * sw.reshape(-1) - m2) + b2
    bias2 = np.tile(bias2, BPC).reshape(64, 1)

    ca_w1 = np.asarray(inp['ca_w1'], np.float32)
    ca_b1 = np.asarray(inp['ca_b1'], np.float32)
    ca_w2 = np.asarray(inp['ca_w2'], np.float32)
    ca_b2 = np.asarray(inp['ca_b2'], np.float32)
    H = ca_w1.shape[0]
    lca1 = np.zeros((2 * C2, BPC * H), np.float32)
    lca2 = np.zeros((BPC * H, 2 * C2), np.float32)
    for b in range(BPC):
        lca1[b * C2:(b + 1) * C2, b * H:(b + 1) * H] = ca_w1.T
        lca2[b * H:(b + 1) * H, b * C2:(b + 1) * C2] = ca_w2.T

    sa_w = np.asarray(inp['sa_w'], np.float32)
    w6 = sa_w[0, :, 1, :]                             # [2 in-ch, 3 dt]
    lsam = np.zeros((64, 6), np.float32)
    lsax = np.zeros((64, 6), np.float32)
    for dt in range(3):
        for b in range(BPC):
            lsam[b * C2:(b + 1) * C2, 2 * dt + b] = w6[0, dt] / C2
            lsax[b * C2, 2 * dt + b] = w6[1, dt]
    sa_g, sa_b, sa_m, sa_v = (float(np.asarray(inp[k]).reshape(-1)[0])
                              for k in ('sa_g', 'sa_b', 'sa_m', 'sa_v'))
    ssa = sa_g / np.sqrt(sa_v + EPS)

    lbch = np.zeros((2, 64), np.float32)
    for b in range(BPC):
        lbch[b, b * C2:(b + 1) * C2] = 0.5

    sep_w = np.asarray(inp['sep_w'], np.float32)[:, 0, 0, :]
    sep_b = np.asarray(inp['sep_b'], np.float32)
    g3, b3, m3, v3 = (np.asarray(inp[k], np.float32) for k in ('g3', 'b3', 'm3', 'v3'))
    s3 = g3 / np.sqrt(v3 + EPS)
    wsep = np.tile(sep_w * s3[:, None] / PW, (BPC, 1))          # [64, 16]
    bsep = np.tile(s3 * (sep_b - m3) + b3, BPC).reshape(64, 1)
    sepd = np.zeros((64, 16 * 64), np.float32)
    for k in range(PW):
        for c in range(64):
            sepd[c, 64 * k + c] = wsep[c, k] / 16.0

    cb = np.zeros((128, CBW), np.float32)
    cb[:, CB_WT:CB_WT + NF] = Wt
    cb[:, CB_KS:CB_KS + 1024] = Kstk.reshape(128, F1 * 2 * C2)
    cb[0:64, CB_LSAM:CB_LSAM + 6] = lsam
    cb[0:64, CB_LSAX:CB_LSAX + 6] = lsax
    cb[0:2, CB_LBC:CB_LBC + 64] = lbch
    cb[0:64, CB_SEP:CB_SEP + 1024] = sepd

    cf = np.zeros((64, CFW), np.float32)
    cf[:, CF_B2H] = 0.5 * bias2[:, 0]
    cf[:, CF_B2X2] = 2.0 * bias2[:, 0]
    cf[0:BPC * H, CF_BCA1] = np.tile(ca_b1, BPC)
    cf[:, CF_BCA2H] = 0.5 * np.tile(ca_b2, BPC)
    cf[0:2, CF_SSAH] = 0.5 * ssa
    cf[0:2, CF_BSAH] = 0.5 * (sa_b - sa_m * ssa)
    cf[:, CF_BSEP] = bsep[:, 0]
    cf[:, CF_BSEP116] = (bsep[:, 0] + 1.0) / 16.0
    cf[:, CF_HALF] = 0.5
    cf[0:64, CF_LCA1:CF_LCA1 + 4] = lca1
    cf[0:4, CF_LCA2:CF_LCA2 + 64] = lca2
    return {'cb': _bf16(cb), 'cf': cf}


def _host_xtiles(x, core):
    xc = np.asarray(x, np.float32)[core * BPC:(core + 1) * BPC, 0]  # [2, C, T]
    xTpad = np.zeros((NBLK * TBLK + 128, BPC * C), np.float32)
    xTpad[31:31 + T, :] = xc.reshape(BPC * C, T).T
    tiles = np.zeros((128, NBLK, BPC * C), np.float32)
    for i in range(NBLK):
        tiles[:, i, :] = xTpad[TBLK * i: TBLK * i + 128]
        tiles[127, i, :] = 1.0
    return _bf16(tiles.reshape(128, NBLK * BPC * C))                # [128, 2048]


def _in_maps(inputs):
    consts = _host_consts(inputs)
    in_maps = []
    for core in range(NCORE):
        m = dict(consts)
        m['xt'] = _host_xtiles(inputs['x'], core)
        in_maps.append(m)
    return in_maps


# ------------------------------------------------------------- device program
_CACHE = {}


def _build_program():
    from concourse import bacc
    nc = bacc.Bacc("TRN2", target_bir_lowering=False, debug=False)
    xt_d = nc.dram_tensor("xt", [128, 2048], BF16, kind="ExternalInput")
    cb_d = nc.dram_tensor("cb", [128, CBW], BF16, kind="ExternalInput")
    cf_d = nc.dram_tensor("cf", [64, CFW], F32, kind="ExternalInput")
    out_d = nc.dram_tensor("out", [BPC, C2, 3], F32, kind="ExternalOutput")

    with tile.TileContext(nc) as tc:
        with (
            tc.tile_pool(name="sb", bufs=1) as sb,
            tc.tile_pool(name="ep", bufs=3) as ep,
            tc.tile_pool(name="qp", bufs=2) as qp,
            tc.tile_pool(name="cpsum", bufs=3, space="PSUM") as cpsum,
            tc.tile_pool(name="dpsum", bufs=2, space="PSUM") as dpsum,
        ):
            xsb = sb.tile([128, 2048], BF16, tag="xt", name="xt_sb")
            cbt = sb.tile([128, CBW], BF16, tag="cb", name="cb_sb")
            cft = sb.tile([64, CFW], F32, tag="cf", name="cf_sb")

            # input DMAs: conv weights via SP hwdge, x head via ACT hwdge,
            # bulky remainder via Pool swdge (Pool is idle early)
            nc.scalar.dma_start(xsb[:, 0:128], xt_d.ap()[:, 0:128])
            nc.sync.dma_start(cbt[:, 0:512], cb_d.ap()[:, 0:512])
            nc.sync.dma_start(cbt[:, 512:NF], cb_d.ap()[:, 512:NF])
            nc.gpsimd.dma_start(xsb[:, 128:384], xt_d.ap()[:, 128:384])
            nc.gpsimd.dma_start(xsb[:, 384:1024], xt_d.ap()[:, 384:1024])
            nc.gpsimd.dma_start(xsb[:, 1024:2048], xt_d.ap()[:, 1024:2048])
            nc.gpsimd.dma_start(cbt[:, NF:CBW], cb_d.ap()[:, NF:CBW])

            wt = cbt[:, CB_WT:CB_WT + NF]
            ks = cbt[:, CB_KS:CB_KS + 1024]
            lsam = cbt[0:64, CB_LSAM:CB_LSAM + 6]
            lsax = cbt[0:64, CB_LSAX:CB_LSAX + 6]
            lbch = cbt[0:2, CB_LBC:CB_LBC + 64]
            sepd = cbt[0:64, CB_SEP:CB_SEP + 1024]
            b2h = cft[:, CF_B2H:CF_B2H + 1]
            b2x2 = cft[:, CF_B2X2:CF_B2X2 + 1]
            bca1 = cft[0:4, CF_BCA1:CF_BCA1 + 1]
            bca2h = cft[:, CF_BCA2H:CF_BCA2H + 1]
            ssah = cft[0:2, CF_SSAH:CF_SSAH + 1]
            bsah = cft[0:2, CF_BSAH:CF_BSAH + 1]
            bsepa = cft[:, CF_BSEP:CF_BSEP + 1]
            bsep116 = cft[:, CF_BSEP116:CF_BSEP116 + 1]
            half = cft[:, CF_HALF:CF_HALF + 1]
            lca1 = cft[:, CF_LCA1:CF_LCA1 + 4]
            lca2 = cft[0:4, CF_LCA2:CF_LCA2 + 64]

            rbuf = sb.tile([128, NBLK * NF], BF16, tag="rb", name="rbuf")
            qbuf = sb.tile([128, NBLK * NF], BF16, tag="qb", name="qbuf")
            h3 = sb.tile([64, T], BF16, tag="h3", name="h3")
            h4p = sb.tile([64, T + 2], BF16, tag="h4p", name="h4p")
            scr = sb.tile([64, T + 2], BF16, tag="scr", name="scr")
            ppad = sb.tile([64, 77], BF16, tag="ppad", name="ppad")
            ones = sb.tile([1, 512], BF16, tag="ones", name="ones")
            nc.gpsimd.memset(ones[:], 1.0)
            halfw = sb.tile([1, 64], BF16, tag="halfw", name="halfw")
            nc.gpsimd.memset(halfw[:], 0.5)
            nc.gpsimd.memset(h4p[:], 0.0)
            nc.gpsimd.memset(scr[:], 0.0)
            nc.gpsimd.memset(ppad[:], 0.0)
            nc.gpsimd.dma_start(cft[:], cf_d.ap())

            casum = [sb.tile([64, 1], F32, tag=f"cas{q}", name=f"cas{q}")
                     for q in range(4)]
            rv = rbuf[:].rearrange("p (blk f toff) -> p f blk toff",
                                   blk=NBLK, f=F1)
            qv = qbuf[:].rearrange("p (blk f toff) -> p f blk toff",
                                   blk=NBLK, f=F1)
            dpt = [None] * 4

            def s2_chunk(q, fp0, fp1):
                if dpt[q] is None:
                    dpt[q] = dpsum.tile([64, 256], F32, tag="dp", name=f"dp{q}")
                for fp in range(fp0, fp1):
                    for n, src_v in enumerate((rv, qv)):
                        nc.tensor.matmul(dpt[q][:], ks[:, 64 * fp:64 * (fp + 1)],
                                         src_v[:, fp, 4 * q:4 * (q + 1), :],
                                         start=(fp == 0 and n == 0),
                                         stop=(fp == F1 - 1 and n == 1))

            def s2_chunk3(part):
                if dpt[3] is None:
                    dpt[3] = dpsum.tile([64, 256], F32, tag="dp", name="dp3")
                for fp in range(F1):
                    for n, src_v in enumerate((rv, qv)):
                        if part == 0:
                            nc.tensor.matmul(dpt[3][:, 0:192],
                                             ks[:, 64 * fp:64 * (fp + 1)],
                                             src_v[:, fp, 12:15, :],
                                             start=(fp == 0 and n == 0),
                                             stop=(fp == F1 - 1 and n == 1))
                        else:
                            nc.tensor.matmul(dpt[3][:, 192:256],
                                             ks[:, 64 * fp:64 * (fp + 1)],
                                             src_v[:, fp, 15:16, :],
                                             start=(fp == 0 and n == 0),
                                             stop=(fp == F1 - 1 and n == 1))

            def s2_drain(q):
                w = 256 if q < 3 else T - 768
                nc.scalar.activation(h3[:, 256 * q:256 * q + w],
                                     dpt[q][:, 0:w], AFT.Copy,
                                     accum_out=casum[q][:])

            def pe_warm(n, tag):
                d = cpsum.tile([64, 256], F32, tag="cp", name=f"warm{tag}")
                for j in range(n):
                    nc.tensor.matmul(d[:], xsb[:, 0:64], xsb[:, 0:256],
                                     start=(j == 0), stop=(j == n - 1))

            hmax0 = sb.tile([64, 1], F32, tag="hm0", name="hmax0")
            hmax1 = sb.tile([64, 1], F32, tag="hm1", name="hmax1")
            hmax2 = sb.tile([64, 1], F32, tag="hm2x", name="hmax2")

            # ---- conv1 + elu(x) = max(x, min(exp x, 1) - 1), 16 blocks
            for i in range(NBLK):
                cp = cpsum.tile([128, NF], F32, tag="cp", name="cp")
                lhs = xsb[:, 128 * i:128 * (i + 1)]
                nc.tensor.matmul(cp[:, 0:512], lhs, wt[:, 0:512])
                nc.tensor.matmul(cp[:, 512:1024], lhs, wt[:, 512:1024])
                if 5 <= i <= 8:
                    s2_chunk(0, 4 * (i - 5), 4 * (i - 4))
                elif 9 <= i <= 12:
                    s2_chunk(1, 4 * (i - 9), 4 * (i - 8))
                elif i >= 13:
                    s2_chunk(2, 4 * (i - 13), 4 * (i - 12))
                o = NF * i
                e_t = ep.tile([128, NF], BF16, tag="e", name="e")
                nc.scalar.activation(e_t[:], cp[:], AFT.Exp)
                if i == 15:
                    nc.scalar.activation(rbuf[:, o:o + NF], cp[:], AFT.Relu)
                    nc.vector.tensor_scalar(qbuf[:, o:o + NF], e_t[:], 1.0,
                                            1.0, op0=AOP.min, op1=AOP.subtract)
                else:
                    nc.vector.tensor_scalar(rbuf[:, o:o + NF], cp[:], 0.0,
                                            None, op0=AOP.max)
                    nc.vector.tensor_scalar(qbuf[:, o:o + QSPL],
                                            e_t[:, 0:QSPL], 1.0, 1.0,
                                            op0=AOP.min, op1=AOP.subtract)
                    nc.gpsimd.tensor_scalar(qbuf[:, o + QSPL:o + NF],
                                            e_t[:, QSPL:NF], 1.0, 1.0,
                                            op0=AOP.min, op1=AOP.subtract)
                if i == 12:
                    s2_drain(0)
                elif i == 13:
                    s2_drain(1)
            s2_chunk(2, 12, F1)
            s2_drain(2)
            nc.vector.tensor_reduce(hmax0[:], h3[:, 0:512],
                                    axis=mybir.AxisListType.X, op=AOP.max)
            nc.vector.tensor_reduce(hmax1[:], h3[:, 512:768],
                                    axis=mybir.AxisListType.X, op=AOP.max)
            s2_chunk3(0)
            s2_chunk3(1)
            s2_drain(3)
            nc.vector.tensor_reduce(hmax2[:], h3[:, 768:T],
                                    axis=mybir.AxisListType.X, op=AOP.max)
            pe_warm(8, "b")

            # ---- channel attention (sigmoid via tanh; biases folded)
            cs01 = sb.tile([64, 1], F32, tag="cs01", name="cs01")
            cs23 = sb.tile([64, 1], F32, tag="cs23", name="cs23")
            cst = sb.tile([64, 1], F32, tag="cst", name="cst")
            nc.vector.tensor_tensor(cs01[:], casum[0][:], casum[1][:], op=AOP.add)
            nc.vector.tensor_tensor(cs23[:], casum[2][:], casum[3][:], op=AOP.add)
            nc.vector.tensor_tensor(cst[:], cs01[:], cs23[:], op=AOP.add)
            hm01 = sb.tile([64, 1], F32, tag="hm01", name="hm01")
            nc.vector.tensor_tensor(hm01[:], hmax0[:], hmax1[:], op=AOP.max)
            hm = sb.tile([64, 1], F32, tag="hmm", name="hm")
            nc.vector.tensor_tensor(hm[:], hm01[:], hmax2[:], op=AOP.max)
            hm2 = sb.tile([64, 1], F32, tag="hm2", name="hm2")
            nc.vector.tensor_scalar(hm2[:], hm[:], 1.0, b2x2, op0=AOP.mult,
                                    op1=AOP.add)
            s3t = sb.tile([64, 1], F32, tag="s3t", name="s3t")
            nc.vector.tensor_scalar(s3t[:], cst[:], 1.0 / T, hm2[:],
                                    op0=AOP.mult, op1=AOP.add)
            z1 = dpsum.tile([4, 1], F32, tag="dp", name="z1")
            nc.tensor.matmul(z1[:], lca1[:], s3t[:])
            ut = sb.tile([4, 1], F32, tag="u", name="u")
            nc.scalar.activation(ut[:], z1[:], AFT.Relu, bias=bca1[:])
            z2 = dpsum.tile([64, 1], F32, tag="dp", name="z2")
            nc.tensor.matmul(z2[:], lca2[:], ut[:])
            t2 = sb.tile([64, 1], F32, tag="t2", name="t2")
            nc.scalar.activation(t2[:], z2[:], AFT.Tanh, bias=bca2h[:], scale=0.5)
            att = sb.tile([64, 1], F32, tag="att", name="att")
            b2a = sb.tile([64, 1], F32, tag="b2a", name="b2a")
            nc.vector.tensor_scalar(att[:], t2[:], 0.5, 0.5, op0=AOP.mult,
                                    op1=AOP.add)
            nc.vector.tensor_scalar(b2a[:], t2[:], b2h, b2h, op0=AOP.mult,
                                    op1=AOP.add)
            # ---- spatial attention: mean/max folded into 3-tap conv matmuls
            msa = sb.tile([2, T], BF16, tag="msa", name="msa")
            for (a, b) in ((0, 500), (500, T)):
                w = b - a
                nc.vector.tensor_scalar(h4p[:, 1 + a:1 + b], h3[:, a:b],
                                        att[:], b2a[:], op0=AOP.mult, op1=AOP.add)
                for bb in range(BPC):
                    nc.gpsimd.partition_all_reduce(
                        scr[32 * bb:32 * (bb + 1), 1 + a:1 + b],
                        h4p[32 * bb:32 * (bb + 1), 1 + a:1 + b],
                        channels=32, reduce_op=bass_isa.ReduceOp.max)
                pp = cpsum.tile([2, 512], F32, tag="cp", name="pp")
                for dt in range(3):
                    nc.tensor.matmul(pp[:, 0:w], lsam[:, 2 * dt:2 * dt + 2],
                                     h4p[:, a + dt:a + dt + w],
                                     start=(dt == 0), stop=False)
                for dt in range(3):
                    nc.tensor.matmul(pp[:, 0:w], lsax[:, 2 * dt:2 * dt + 2],
                                     scr[:, a + dt:a + dt + w],
                                     start=False, stop=(dt == 2))
                nc.scalar.activation(msa[:, a:b], pp[:, 0:w], AFT.Tanh,
                                     bias=bsah[:], scale=ssah[:])
                pe_warm(3, f"c{a}")

            # ---- h5 = h4 * (0.5 + 0.5 tanh), elu, pool(16)
            h5 = sb.tile([64, 992], BF16, tag="h5", name="h5")
            e5 = sb.tile([64, 992], BF16, tag="e5", name="e5")
            q5 = sb.tile([64, 992], BF16, tag="q5", name="q5")
            l5 = sb.tile([64, 992], BF16, tag="l5", name="l5")
            for (a, b) in ((0, 500), (500, 992)):
                w = b - a
                bp = cpsum.tile([64, 512], F32, tag="cp", name="bp")
                nc.tensor.matmul(bp[:, 0:w], lbch[:], msa[:, a:b], start=True,
                                 stop=False)
                nc.tensor.matmul(bp[:, 0:w], halfw[:], ones[:, 0:w],
                                 start=False, stop=True)
                nc.vector.tensor_tensor(h5[:, a:b], h4p[:, 1 + a:1 + b],
                                        bp[:, 0:w], op=AOP.mult)
                nc.scalar.activation(e5[:, a:b], h5[:, a:b], AFT.Exp)
                nc.vector.tensor_scalar(q5[:, a:b], e5[:, a:b], 1.0, 1.0,
                                        op0=AOP.min, op1=AOP.subtract)
            for h, (a, b) in enumerate(((0, 496), (496, 992))):
                nc.vector.tensor_tensor(l5[:, a:b], h5[:, a:b], q5[:, a:b],
                                        op=AOP.max)
                with nc.allow_low_precision(reason="16-wide pool, 2e-2 budget"):
                    nc.vector.tensor_reduce(
                        ppad[:, 7 + 31 * h:7 + 31 * (h + 1)],
                        l5[:, a:b].rearrange("p (w k) -> p w k", k=16),
                        axis=mybir.AxisListType.X, op=AOP.add)

            # ---- separable temporal conv via 16 accumulating diag matmuls
            pe_warm(8, "d")
            sp6 = cpsum.tile([64, 62], F32, tag="cp", name="sp6")
            for k in range(PW):
                nc.tensor.matmul(sp6[:], sepd[:, 64 * k:64 * (k + 1)],
                                 ppad[:, k:k + 62],
                                 start=(k == 0), stop=(k == PW - 1))

            # ---- final elu + pool(16), all pre-scaled by 1/16 in sepd:
            # e6 = exp(16*sp6 + bsep) from PSUM; s6 = sp6 + (bsep+1)/16;
            # l6 = max(s6, min(e6,1)/16-ish) = elu(sep+bsep)/16 + 1/16
            s6 = sb.tile([64, 62], BF16, tag="s6", name="s6")
            nc.vector.tensor_scalar(s6[:], sp6[:], 1.0, bsep116[:],
                                    op0=AOP.mult, op1=AOP.add)
            e6 = sb.tile([64, 62], BF16, tag="e6", name="e6")
            nc.scalar.activation(e6[:], sp6[:], AFT.Exp, bias=bsepa[:],
                                 scale=16.0)
            q6 = sb.tile([64, 62], BF16, tag="q6", name="q6")
            nc.vector.tensor_scalar(q6[:], e6[:], 1.0 / 16.0, 1.0 / 16.0,
                                    op0=AOP.mult, op1=AOP.min)
            l6 = sb.tile([64, 48], BF16, tag="l6", name="l6")
            nc.vector.tensor_tensor(l6[:], s6[:, 0:48], q6[:, 0:48], op=AOP.max)
            p3 = sb.tile([64, 3], F32, tag="p3", name="p3")
            nc.vector.tensor_reduce(p3[:],
                                    l6[:].rearrange("p (w k) -> p w k", k=16),
                                    axis=mybir.AxisListType.X, op=AOP.add)
            ot = sb.tile([64, 3], F32, tag="ot", name="ot")
            nc.vector.tensor_scalar(ot[:], p3[:], -1.0, None, op0=AOP.add)
            nc.sync.dma_start(out_d.ap().rearrange("a b c -> (a b) c"), ot[:])
    nc.compile()
    return nc


def kernel(**inputs):
    if 'nc' not in _CACHE:
        _CACHE['nc'] = _build_program()
    nc = _CACHE['nc']
    res = run_bass_kernel_spmd(nc, _in_maps(inputs), list(range(NCORE)))
    out = np.concatenate([np.asarray(res.results[i]['out'])
                          for i in range(NCORE)], axis=0)
    return out.astype(np.float32)
